# revision 21
# baseline (speedup 1.0000x reference)
"""Trainium2 Bass kernel for nn_Pooling_block (B=128, N=785, C=384, pp=2).

Pure data-parallel over batch: 16 batches per core x 8 NeuronCores.

v3: channel-major node pipeline, batch-PAIR processing.
  - Host pre-casts to fp16 and ships x-nodes CHANNEL-MAJOR
    (xc [NB, 3, 128, 784]); edge row-major fp16. Halves HBM reads vs f32.
  - Per pair of batches (8 pairs/core):
      edge mean: fp16 fold tile [128, 2b, 8, 384] -> add-tree (gpsimd+DVE)
        -> 1 ones-matmul per batch -> sigmoid rows -> 3 tiny transpose MMs
        per batch into a [128, 3, 2] column.
      node mean: ONE grouped DVE reduce over xc (fp16 accum, sigmoid
        tolerant) -> sigmoid column.
      ci chain in f32r: cirow MMs (3/batch) -> row->col tiny MMs -> fp16
        ci column [128, 3, 2b].
      scores on PE: ci column as stationary against xc moving ([1, 392]
        PSUM halves) -> sigmoid rows -> sp=(sg+1)*wpat (one STT) ->
        strided t01row -> PE broadcast -> T4 [128, 2b, 2q, 196].
      A pair-sums c-major: one rank-5 strided DVE add per cch.
      pooled = A*T4 (mul) + q-fold (add) per cch, directly c-major with a
        cls column -> NO transposes.
      final: 6 MMs (fp16) per (batch, row-chunk) into a [128, 2, 384] PSUM
        tile -> ONE [rn, 768] copy -> store (issued on gpsimd SWDGE).
  - All loads on the Sync HWDGE queue; stores on gpsimd; edge fold-pad
    memset amortized once per pool buffer.
"""
import os
import sys

sys.path.insert(0, "/opt/trn_rl_repo")

import numpy as np

import concourse.bass as bass
import concourse.tile as tile
from concourse import bacc, mybir
from concourse.bass_utils import run_bass_kernel_spmd

B, N, C = 128, 785, 384
HW = N - 1          # 784
H = 28              # grid side
HP = 14             # pooled grid side
NPATCH = HP * HP    # 196
NB = 16             # batches per core
NPAIR = NB // 2
NCORES = 8
NOUT = 1 + NPATCH   # 197
CO = 2 * C          # 768

F32 = mybir.dt.float32
F32R = mybir.dt.float32r
FP16 = mybir.dt.float16
ADD = mybir.AluOpType.add
MUL = mybir.AluOpType.mult
SIGMOID = mybir.ActivationFunctionType.Sigmoid
COPY = mybir.ActivationFunctionType.Copy
AXIS_X = mybir.AxisListType.X

KSTAGE = int(os.environ.get("KSTAGE", "99"))
EDP_BUFS = 2


def build_program(w_scalars):
    """w_scalars = (w00, w01, w10, w11) when the per-patch weights are
    channel-uniform, else None (general channel-varying path)."""
    nc = bacc.Bacc(None, target_bir_lowering=False, debug=False)

    xc_d = nc.declare_dram_parameter("xc", [3, 128, NB, HW], FP16, isOutput=False)
    e_d = nc.declare_dram_parameter("edge", [NB, N, C], FP16, isOutput=False)
    wlt_d = nc.declare_dram_parameter("wlt", [C, C], F32, isOutput=False)
    wct_d = nc.declare_dram_parameter("wct", [C, CO], FP16, isOutput=False)
    clsc_d = nc.declare_dram_parameter("cls_cm", [128, 3, NB], FP16, isOutput=False)
    if w_scalars is None:
        wqr_d = nc.declare_dram_parameter("wqr", [128, 3, 2, 2], FP16, isOutput=False)
    out_d = nc.declare_dram_parameter("out", [NB, NOUT, CO], F32, isOutput=True)

    with tile.TileContext(nc) as tc:
        with (
            tc.tile_pool(name="const", bufs=1) as cpool,
            tc.tile_pool(name="gx", bufs=4) as gxp,
            tc.tile_pool(name="ed", bufs=EDP_BUFS) as edp,
            tc.tile_pool(name="work", bufs=3) as wk,
            tc.tile_pool(name="small", bufs=3) as sm,
            tc.tile_pool(name="ost", bufs=2) as ostp,
            tc.tile_pool(name="psA", bufs=1, space="PSUM") as psA,
            tc.tile_pool(name="psF", bufs=2, space="PSUM") as psF,
        ):
            # ---- constants ----
            ones_e = cpool.tile([128, 1], FP16)
            nc.vector.memset(ones_e[:], 1.0 / N)
            one_h11 = cpool.tile([1, 1], FP16)
            nc.vector.memset(one_h11[:], 1.0)
            ones_row_h = cpool.tile([1, 128], FP16)
            nc.vector.memset(ones_row_h[:], 1.0)

            wlt_f = cpool.tile([128, 3, C], F32, tag="wltf")
            nc.sync.dma_start(
                wlt_f[:], wlt_d.rearrange("(k p) c -> p k c", k=3, p=128)
            )
            wlt_r = cpool.tile([128, 3, C], F32R, tag="wltr")
            nc.vector.tensor_copy(wlt_r[:], wlt_f[:])

            wct_h = cpool.tile([128, 3, CO], FP16, tag="wcth")
            nc.sync.dma_start(
                wct_h[:], wct_d.rearrange("(k p) co -> p k co", k=3, p=128)
            )
            cls_h = cpool.tile([128, 3, NB], FP16)
            nc.sync.dma_start(cls_h[:], clsc_d[:])

            # wpat[n] = w[q, r] for node n = 56i + 28q + 2j + r (uniform case)
            wpat = None
            if w_scalars is not None and len(set(w_scalars)) > 1:
                wpat = cpool.tile([1, HW], FP16)
                wpv = wpat[:].rearrange(
                    "o (i q j r) -> o q r i j", i=HP, q=2, j=HP, r=2
                )
                for q in range(2):
                    for r in range(2):
                        nc.vector.memset(
                            wpv[:, q, r], float(w_scalars[2 * q + r])
                        )
            unit_w = w_scalars is not None and wpat is None and w_scalars[0] == 1.0
            scale_w = (
                None if (w_scalars is None or wpat is not None or unit_w)
                else float(w_scalars[0])
            )
            wqr_t = None
            if w_scalars is None:
                wqr_t = cpool.tile([128, 3, 2, 2], FP16, tag="wqr")
                nc.sync.dma_start(wqr_t[:], wqr_d[:])

            # ---- per-pair pipeline ----
            for bp in range(NPAIR):
                b0 = 2 * bp
                # -- loads --
                xc = gxp.tile([128, 3, 2, HW], FP16, tag="xc")
                for cch in range(3):
                    nc.sync.dma_start(xc[:, cch, :, :], xc_d[cch, :, b0 : b0 + 2, :])

                ef = edp.tile([128, 2, 8, C], FP16, tag="ef")
                if bp < EDP_BUFS:
                    nc.vector.memset(ef[:, :, 6:8, :], 0.0)
                for bi in range(2):
                    nc.sync.dma_start(
                        ef[:, bi, 0:6, :],
                        e_d[b0 + bi, 0:768, :].rearrange(
                            "(p k) c -> p k c", p=128, k=6
                        ),
                    )
                    nc.sync.dma_start(ef[0:17, bi, 6, :], e_d[b0 + bi, 768:785, :])

                if KSTAGE < 2:
                    continue
                # -- edge mean: add-tree; gpsimd takes the wide first level --
                e4 = wk.tile([128, 2, 4, C], FP16, tag="e4")
                nc.gpsimd.tensor_add(e4[:], ef[:, :, 0:4, :], ef[:, :, 4:8, :])
                e2 = wk.tile([128, 2, 2, C], FP16, tag="e2")
                nc.vector.tensor_add(e2[:], e4[:, :, 0:2, :], e4[:, :, 2:4, :])
                e1 = wk.tile([128, 2, C], FP16, tag="e1")
                nc.vector.tensor_add(e1[:], e2[:, :, 0, :], e2[:, :, 1, :])

                se_sb = []
                for bi in range(2):
                    es = psA.tile([1, C], F32, tag="stat")
                    nc.tensor.matmul(
                        es[:], ones_e[:], e1[:, bi, :], start=True, stop=True
                    )
                    se = sm.tile([1, C], FP16, tag=f"se{bi}")
                    nc.scalar.activation(se[:], es[:], SIGMOID)
                    se_sb.append(se)

                # -- node mean: grouped fp16 reduce over xc + sigmoid --
                nsum = sm.tile([128, 3, 2], FP16, tag="nsum")
                with nc.allow_low_precision("node-mean fp16 accum feeds sigmoid"):
                    nc.vector.tensor_reduce(
                        nsum[:], xc[:], axis=AXIS_X, op=ADD
                    )
                sn_col = sm.tile([128, 3, 2], FP16, tag="sncol")
                nc.scalar.activation(sn_col[:], nsum[:], SIGMOID, scale=1.0 / HW)

                if KSTAGE < 3:
                    continue
                # -- s column; ci chain (f32r) --
                secol = psA.tile([128, 3, 2], F32, tag="secol")
                for bi in range(2):
                    for cch in range(3):
                        nc.tensor.matmul(
                            secol[:, cch, bi : bi + 1],
                            se_sb[bi][:, 128 * cch : 128 * (cch + 1)],
                            one_h11[:], start=True, stop=True,
                        )
                s_col = sm.tile([128, 3, 2], F32R, tag="scol")
                nc.vector.tensor_add(s_col[:], secol[:], sn_col[:])

                ci_h = sm.tile([128, 3, 2], FP16, tag="cih")
                cicol = psA.tile([128, 3, 2], F32, tag="cicol")
                for bi in range(2):
                    cirp = psA.tile([1, C], F32, tag="stat")
                    for cch in range(3):
                        nc.tensor.matmul(
                            cirp[:], s_col[:, cch, bi : bi + 1], wlt_r[:, cch, :],
                            start=(cch == 0), stop=(cch == 2),
                        )
                    ci_sb = sm.tile([1, C], FP16, tag=f"cisb{bi}")
                    nc.scalar.copy(ci_sb[:], cirp[:])
                    for cch in range(3):
                        nc.tensor.matmul(
                            cicol[:, cch, bi : bi + 1],
                            ci_sb[:, 128 * cch : 128 * (cch + 1)],
                            one_h11[:], start=True, stop=True,
                        )
                nc.scalar.copy(ci_h[:], cicol[:])

                if KSTAGE < 4:
                    continue
                # -- scores on PE; sp row; T build + broadcast --
                sg = sm.tile([1, 2, HW], F32, tag="sg")
                for bi in range(2):
                    for h0, hn in ((0, 392), (392, 392)):
                        scp = psA.tile([128, 392], F32, tag="tb")
                        for cch in range(3):
                            nc.tensor.matmul(
                                scp[0:1, :],
                                ci_h[:, cch, bi : bi + 1],
                                xc[:, cch, bi, h0 : h0 + hn],
                                start=(cch == 0), stop=(cch == 2),
                            )
                        nc.scalar.activation(
                            sg[:, bi, h0 : h0 + hn], scp[0:1, :], SIGMOID
                        )

                sp = sm.tile([1, 2, HW], FP16, tag="sp")
                if wpat is not None:
                    wpb = wpat[:].rearrange("o (b n) -> o b n", b=1).broadcast_to(
                        (1, 2, HW)
                    )
                    nc.vector.scalar_tensor_tensor(
                        sp[:], sg[:], 1.0, wpb, ADD, MUL
                    )
                else:
                    nc.vector.tensor_scalar_add(sp[:], sg[:], 1.0)

                t4g = None
                if w_scalars is not None:
                    # t01row[b, q, ij] = sp[b, n(q,r=0,ij)] + sp[b, n(q,r=1,ij)]
                    spv = sp[:].rearrange(
                        "o b (i q j r) -> o b q i j r", i=HP, q=2, j=HP, r=2
                    )
                    t01 = sm.tile([1, 2, 2, NPATCH], FP16, tag="t01")
                    t01v = t01[:].rearrange("o b q (i j) -> o b q i j", i=HP, j=HP)
                    for bi in range(2):
                        nc.vector.tensor_add(
                            t01v[:, bi], spv[:, bi, :, :, :, 0],
                            spv[:, bi, :, :, :, 1],
                        )
                    # broadcast T rows to 128 partitions, per batch
                    t4 = sm.tile([128, 2, 2, NPATCH], FP16, tag="t4")
                    for bi in range(2):
                        tbp = psA.tile([128, 392], F32, tag="tb")
                        nc.tensor.matmul(
                            tbp[:], ones_row_h[:],
                            t01[:, bi, :, :].rearrange("o q n -> o (q n)"),
                            start=True, stop=True,
                        )
                        t4o = t4[:, bi, :, :].rearrange("p q n -> p (q n)")
                        if scale_w is not None:
                            nc.scalar.activation(t4o, tbp[:], COPY, scale=scale_w)
                        else:
                            nc.scalar.copy(t4o, tbp[:])
                else:
                    # general channel-varying weights: T4[c, b, q, n] =
                    #   w[c,q,0]*(sp[b,n(q,0)]) + w[c,q,1]*(sp[b,n(q,1)])
                    spv = sp[:].rearrange(
                        "o b (i q j r) -> o b q r i j", i=HP, q=2, j=HP, r=2
                    )
                    spb = sm.tile([128, 2, 2, 2, NPATCH], FP16, tag="spb")
                    for bi in range(2):
                        for q in range(2):
                            for r in range(2):
                                tbp = psA.tile([128, 392], F32, tag="tb")
                                nc.tensor.matmul(
                                    tbp[:, 0:NPATCH], ones_row_h[:],
                                    spv[:, bi, q, r],
                                    start=True, stop=True,
                                )
                                nc.scalar.copy(spb[:, bi, q, r, :], tbp[:, 0:NPATCH])
                    t4 = None
                    t4g = []
                    for cch in range(3):
                        w0 = wqr_t[:, cch, :, 0:1].rearrange(
                            "p q (b o) -> p b q o", b=1, o=1
                        ).broadcast_to((128, 2, 2, NPATCH))
                        w1 = wqr_t[:, cch, :, 1:2].rearrange(
                            "p q (b o) -> p b q o", b=1, o=1
                        ).broadcast_to((128, 2, 2, NPATCH))
                        ta = sm.tile([128, 2, 2, NPATCH], FP16, tag=f"t4a{cch}")
                        nc.vector.tensor_mul(ta[:], spb[:, :, :, 0, :], w0)
                        tg = sm.tile([128, 2, 2, NPATCH], FP16, tag=f"t4g{cch}")
                        tb_ = sm.tile([128, 2, 2, NPATCH], FP16, tag=f"t4b{cch}")
                        nc.vector.tensor_mul(tb_[:], spb[:, :, :, 1, :], w1)
                        nc.vector.tensor_add(tg[:], ta[:], tb_[:])
                        t4g.append(tg)

                if KSTAGE < 5:
                    continue
                # -- A pair-sums + pooled, c-major --
                pooled = []
                for cch in range(3):
                    # A_q[b, ij] = x[b, 56i+2j+q] + x[b, 56i+28+2j+q]
                    # (hh = vertical position inside the 2x2 patch)
                    xv = xc[:, cch, :, :].rearrange(
                        "p b (i hh j q) -> p b hh i j q", i=HP, hh=2, j=HP, q=2
                    )
                    a_c = wk.tile([128, 2, 2, NPATCH], FP16, tag=f"ac{cch}")
                    # out iterates (i, j, q) to match the input views
                    av = a_c[:].rearrange(
                        "p b q (i j) -> p b i j q", i=HP, j=HP
                    )
                    for bi in range(2):
                        nc.vector.tensor_add(
                            av[:, bi], xv[:, bi, 0], xv[:, bi, 1]
                        )
                    m_c = wk.tile([128, 2, 2, NPATCH], FP16, tag=f"mc{cch}")
                    tsel = t4 if w_scalars is not None else t4g[cch]
                    nc.gpsimd.tensor_mul(m_c[:], a_c[:], tsel[:])
                    pc = wk.tile([128, 2, NOUT], FP16, tag=f"pc{cch}")
                    nc.gpsimd.tensor_add(
                        pc[:, :, 1:NOUT], m_c[:, :, 0, :], m_c[:, :, 1, :]
                    )
                    nc.scalar.copy(
                        pc[:, :, 0:1],
                        cls_h[:, cch, b0 : b0 + 2].rearrange(
                            "p (b o) -> p b o", b=2, o=1
                        ),
                    )
                    pooled.append(pc)

                if KSTAGE < 6:
                    continue
                # -- final matmul (fp16): [cls|pooled].T @ W_out_cls.T --
                for bi in range(2):
                    for rch, (r0, rn) in enumerate(((0, 128), (128, 69))):
                        # [128, 2, 512] so each nh half is PSUM-bank aligned
                        fo = psF.tile([128, 2, 512], F32, tag="fo")
                        for nh in range(2):
                            for cch in range(3):
                                nc.tensor.matmul(
                                    fo[0:rn, nh, 0:C],
                                    pooled[cch][:, bi, r0 : r0 + rn],
                                    wct_h[:, cch, C * nh : C * (nh + 1)],
                                    start=(cch == 0), stop=(cch == 2),
                                )
                        stile = ostp.tile([128, CO], F32, tag=f"ost{rch}")
                        nc.scalar.copy(
                            stile[0:rn, :].rearrange("p (n c) -> p n c", n=2),
                            fo[0:rn, :, 0:C],
                        )
                        nc.gpsimd.dma_start(
                            out_d[b0 + bi, r0 : r0 + rn, :], stile[0:rn, :]
                        )

    nc.compile()
    return nc


def prep_inputs(x, edge, W_lin, W_out_cls, weights):
    """Returns (w_scalars, in_maps) shared by kernel() and test harness."""
    x = np.asarray(x, dtype=np.float32)
    edge = np.asarray(edge, dtype=np.float32)
    wlt = np.ascontiguousarray(np.asarray(W_lin).T, dtype=np.float32)
    wct = np.ascontiguousarray(np.asarray(W_out_cls).T, dtype=np.float16)
    w = np.asarray(weights, dtype=np.float32)

    c_uniform = bool(np.all(w == w[0:1]))
    w_scalars = tuple(float(v) for v in w[0].reshape(4)) if c_uniform else None

    x16 = x.astype(np.float16)
    in_maps = []
    for core in range(NCORES):
        sl = slice(core * NB, (core + 1) * NB)
        cls_cm = np.ascontiguousarray(
            x[sl, 0, :].T.reshape(3, 128, NB).transpose(1, 0, 2), dtype=np.float16
        )
        xc = np.ascontiguousarray(
            x16[sl, 1:, :].transpose(2, 0, 1).reshape(3, 128, NB, HW)
        )
        m = {
            "xc": xc,
            "edge": np.ascontiguousarray(edge[sl], dtype=np.float16),
            "wlt": wlt, "wct": wct, "cls_cm": cls_cm,
        }
        if w_scalars is None:
            m["wqr"] = np.ascontiguousarray(
                np.broadcast_to(w.reshape(3, 128, 2, 2), (3, 128, 2, 2))
                .transpose(1, 0, 2, 3), dtype=np.float16
            )
        in_maps.append(m)
    return w_scalars, in_maps


def kernel(x, edge, W_lin, W_out_cls, weights):
    w_scalars, in_maps = prep_inputs(x, edge, W_lin, W_out_cls, weights)
    nc = build_program(w_scalars)
    res = run_bass_kernel_spmd(nc, in_maps, list(range(NCORES)))
    out = np.concatenate([r["out"] for r in res.results], axis=0)
    return out
